# revision 40
# baseline (speedup 1.0000x reference)
"""Trainium2 Bass kernel for nn_ButterflyRotation (B=8192, D=4096, L=12).

Strategy (pure data parallel over 8 cores, 1024 batch rows each):

The 12 butterfly layers factor as T = T2 . T1 where
  - T1 (layers 0-6, strides 1..64) is block-diagonal over 32 outer blocks:
    a 128x128 rotation A_o acting on the inner index q = j[6:0].
  - T2 (layers 7-11, strides 128..2048) mixes only the outer index
    o = j[11:7] (32 values) with coefficients depending on q: for each q a
    32x32 matrix B_q. Packed 4-per-128-partitions as block-diagonal 128x128
    matrices WB_v over partitions p = j[6:5]*32 + o, one per v = j[4:0].

Pipeline (bf16 datapath, rel-err budget 2e-2 >> bf16's ~2e-3):
  - host casts x to bf16: input DRAM traffic halves to 8 MiB/core.
  - input lands d-major directly via the DMA XBAR transpose (16x128-tile
    stream transpose, 2-byte dtype): no PE transposes, no PSUM evacs on
    the input path. One XBAR DMA per 128-wide o-block over the full 1024
    core batch (32 total) keeps the HWDGE descriptor-gen cost small;
    they are emitted just-in-time (2 o4-blocks lookahead) so z1 stays
    small and the XBARs stream continuously across reps.
  - stage A: bf16 matmuls (1 cyc/row at any N, vs 4 for fp32), 4
    o-blocks x 256 batch per 2-bank PSUM tile so evacuation is one
    [128,1024] ACT copy (ACT runs ONLY these: phase-pure queue, no
    head-blocking by tail work).
  - mid-pipeline j[4:0] <-> j[11:7] swap: DVE 32x32 stream transpose
    (bf16 SBUF->SBUF, no fast mode exists -> 35.7us/rep, DVE-only).
    This is the only partition-crossing step; GPSIMD cannot read PSUM
    and the XBAR only does full 128-wide transposes, so it must be DVE.
  - stage B: bf16 matmuls, lhsT = the data so output lands batch-major.
    8 matmuls per 2-bank PSUM tile -> one [128,1024] evac, scattered
    into the natural d order, cast to fp16. B-evacs split ACT 18 / DVE
    14 per rep to balance both copy engines at ~52us.
  - full-rep software pipelining: each rep's swap/stage-B/store tail is
    drained piece-by-piece through the NEXT rep's stage-A schedule (z2
    slabs live 2 reps = 128KB/partition), so every drained piece's data
    is long ready and the in-order engine queues never head-block.
  - output ships fp16 (host upcasts): 8 MiB/core out.
  - output DMA + weight loads ride the GPSIMD software DGE so the sync
    HWDGE queue only carries the latency-critical XBAR input loads.

Engine budget per rep (1024 rows), cost-model estimates: DMA ~52us
(XBAR in 28.7 + out 23.3), ACT ~52us, DVE ~52us, PE ~30us, HWDGE ~26us.
TimelineSim steady-state slope: ~89us/rep; HW paired-slope measures
~58-76us/rep (axon-tunnel timing noise is large).
End-to-end rel l2 error vs the fp32 reference: ~3e-3 (bf16 rounding of
x, z2/z3 and the two weight stages; fp16 output adds ~3e-4).
"""

from contextlib import ExitStack

import numpy as np

import concourse.bass as bass  # noqa: F401 (kept for clarity)
import concourse.tile as tile
from concourse import bacc, mybir
from concourse import bass_utils

F32 = mybir.dt.float32
F32R = mybir.dt.float32r
F16 = mybir.dt.float16
BF16 = mybir.dt.bfloat16

DIM = 4096
LAYERS = 12
BATCH = 8192
N_CORES = 8
BC = BATCH // N_CORES          # 1024 batch rows per core
GROUP = 128                    # batch rows per pipeline group
NGRP = BC // GROUP             # 8
NB_O = 32                      # outer blocks j[11:7]
NQ = 128                       # inner j[6:0]

_cache = {}


# ---------------------------------------------------------------- host math
def _apply_layers(x, angles, layers):
    B, d = x.shape
    out = x
    for l in layers:
        stride = 1 << l
        nb = d // (2 * stride)
        theta = angles[l].reshape(nb, stride)
        c = np.cos(theta)
        s = np.sin(theta)
        o = out.reshape(B, nb, 2, stride)
        xl = o[:, :, 0, :]
        xr = o[:, :, 1, :]
        new_l = c * xl + s * xr
        new_r = -s * xl + c * xr
        out = np.stack([new_l, new_r], axis=2).reshape(B, d)
    return out


def _build_weights(angles):
    """WA[o][q,q'] = lhsT for stage A; WBblk[j65,v] = 32x32 lhsT blocks."""
    a64 = angles.astype(np.float64)
    I = np.eye(DIM, dtype=np.float64)
    M1 = _apply_layers(I, a64, range(0, 7))     # = T1^T (block diagonal)
    M2 = _apply_layers(I, a64, range(7, 12))    # = T2^T (q-diagonal)

    WA = np.zeros((NB_O, NQ, NQ), dtype=np.float32)
    for o in range(NB_O):
        WA[o] = M1[o*128:(o+1)*128, o*128:(o+1)*128].astype(np.float32)

    # WB is block-diagonal: ship only the nonzero 32x32 blocks.
    WBblk = np.zeros((4, 32, 32, 32), dtype=np.float32)
    for j65 in range(4):
        for v in range(32):
            q = j65 * 32 + v
            WBblk[j65, v] = M2[q::128, q::128].astype(np.float32)
    return WA, WBblk


# ---------------------------------------------------------------- device IR
def _build_program(reps=1):
    nc = bacc.Bacc("TRN2", target_bir_lowering=False, debug=False,
                   num_devices=N_CORES)
    x_d = nc.dram_tensor("x", [BC, DIM], BF16, kind="ExternalInput").ap()
    wa_d = nc.dram_tensor("wa", [NB_O, 128, 128], BF16,
                          kind="ExternalInput").ap()
    wb_d = nc.dram_tensor("wb", [4, 32, 32, 32], BF16,
                          kind="ExternalInput").ap()
    id_d = nc.dram_tensor("ident", [128, 128], F32R,
                          kind="ExternalInput").ap()
    out_d = nc.dram_tensor("out", [BC, DIM], F16, kind="ExternalOutput").ap()

    with tile.TileContext(nc, trace_sim=False) as tc, ExitStack() as ctx:
        wpool = ctx.enter_context(tc.tile_pool(name="w", bufs=1))
        z1pool = ctx.enter_context(tc.tile_pool(name="z1", bufs=14))
        z2pool = ctx.enter_context(tc.tile_pool(name="z2", bufs=8))
        z3pool = ctx.enter_context(tc.tile_pool(name="z3", bufs=2))
        opool = ctx.enter_context(tc.tile_pool(name="xout", bufs=2))
        pa = ctx.enter_context(tc.tile_pool(name="pa", bufs=2, space="PSUM"))
        pb = ctx.enter_context(tc.tile_pool(name="pb", bufs=2, space="PSUM"))

        wa_sb = wpool.tile([128, NB_O * 128], BF16, tag="wa")
        wb_sb = wpool.tile([128, 32 * 128], BF16, tag="wb")
        ident = wpool.tile([128, 128], F32R, tag="ident")
        nc.gpsimd.dma_start(ident[:], id_d[:])
        # wa in quarters so stage A's first matmuls only wait on 0.25 MiB
        for k in range(4):
            nc.gpsimd.dma_start(
                wa_sb[:].rearrange("q (o m) -> q o m", m=128)[:, 8*k:8*k+8],
                wa_d[8*k:8*k+8].rearrange("o q m -> q o m"))
        # wb_sb is block-diagonal: zero it once, then land only the 32x32
        # blocks (4 DMAs, one per partition quarter j65)
        nc.gpsimd.memset(wb_sb[:], 0.0)
        for j65 in range(4):
            dst = wb_sb[j65*32:(j65+1)*32, :].rearrange(
                "o (v m) -> o v m", m=128)[:, :, j65*32:(j65+1)*32]
            nc.gpsimd.dma_start(dst, wb_d[j65].rearrange("v o m -> o v m"))

        # HAM warm-up: dummy matmuls during the otherwise-idle DMA head so
        # the PE clock-gate is at 2.4 GHz when real work arrives
        for i in range(28):
            pw = pb.tile([128, 128], F32, tag="pb", name=f"warm_{i}")
            nc.tensor.matmul(pw[:], ident[:], ident[:])

        z1tiles = {}              # (rep, o) -> tile handle

        def emit_xbar_block(r, o4):
            """XBAR-load o-blocks o4*4..o4*4+3 of rep r (full core batch).

            DMA XBAR transpose: x[0:1024, o*128:(o+1)*128] ->
            [128 q-partitions, 1024 batch].
            """
            if r >= reps:
                return
            for k in range(4):
                o = o4 * 4 + k
                t = z1pool.tile([128, BC], BF16, tag="z1",
                                name=f"z1_{r}_{o}")
                nc.sync.dma_start(t[:], x_d[:, o*128:(o+1)*128],
                                  transpose=True)
                z1tiles[(r, o)] = t

        def make_tail(r, g, z2_slab):
            """Tail work for 128-row half-slab g: swap, stage B, evac, store.

            z2_slab covers a 256-row group pair; half-slab g reads rows
            (g%2)*128..+128 of it. Returned as closures, drained across
            the NEXT rep's whole stage-A schedule (full-rep software
            pipelining: by drain time every piece's data is long ready,
            so the in-order engine queues never head-block).
            """
            state = {}
            half = (g % 2) * 4096

            def ensure():
                if "z3" not in state:
                    state["z3"] = z3pool.tile([128, GROUP * 32], BF16,
                                              tag="z3", name=f"z3_{r}_{g}")
                    state["xo"] = opool.tile([128, DIM], F16, tag="xo",
                                             name=f"xo_{r}_{g}")

            def do_swap(s):
                ensure()
                # 32x32 stream transpose on b-32 range s:
                # [q=(j65,v),(b,o)] -> [(j65,o),(b,v)]
                sl = slice(half + s * 1024, half + (s + 1) * 1024)
                dl = slice(s * 1024, (s + 1) * 1024)
                nc.vector.transpose(state["z3"][:, dl], z2_slab[:, sl])

            def do_b(vq):
                z3v = state["z3"][:].rearrange("p (b v) -> p b v", v=32)
                ps_b = pb.tile([128, 8 * 128], F32, tag="pb")
                for vv in range(8):
                    v = vq * 8 + vv
                    nc.tensor.matmul(ps_b[:, vv*128:(vv+1)*128],
                                     z3v[:, :, v],
                                     wb_sb[:, v*128:(v+1)*128])
                # evac scatter: out free j' = o'*128 + j65*32 + v
                dst = state["xo"][:].rearrange(
                    "b (o f v) -> b v f o", f=4, v=32)[
                    :, vq*8:(vq+1)*8, :, :]
                src = ps_b[:].rearrange("b (v f o) -> b v f o", v=8, f=4)
                # balance tail copies: DVE has the swaps (35.7us/rep), so
                # ACT takes 18 of 32 B-evacs, DVE 14 -> ~52us each
                if vq == 1 or (vq == 3 and g >= 2):
                    nc.vector.tensor_copy(dst, src)
                else:
                    nc.scalar.copy(dst, src)

            def do_store():
                row0 = g * GROUP
                nc.gpsimd.dma_start(out_d[row0:row0 + GROUP, :],
                                    state["xo"][:])

            pieces = [lambda s=s: do_swap(s) for s in range(4)]
            pieces += [lambda vq=vq: do_b(vq) for vq in range(3)]
            pieces.append(lambda: (do_b(3), do_store()))
            return pieces

        # prologue: first two XBAR blocks of rep 0
        emit_xbar_block(0, 0)
        emit_xbar_block(0, 1)

        tails = []                # pending tail pieces from rep r-1
        for r in range(reps):
            # 4 slabs per rep, each covering a 256-row group pair
            z2_slabs = [z2pool.tile([128, 2 * GROUP * 32], BF16, tag="z2",
                                    name=f"z2_{r}_{gp}") for gp in range(4)]
            drain_credit = 0.0
            for o4 in range(8):
                # JIT XBAR lookahead: two o4-blocks ahead
                if o4 < 6:
                    emit_xbar_block(r, o4 + 2)
                else:
                    emit_xbar_block(r + 1, o4 - 6)
                for gp in range(4):
                    ps_a = pa.tile([128, 1024], F32, tag="pa")
                    for k in range(4):
                        o = o4 * 4 + k
                        nc.tensor.matmul(
                            ps_a[:, k*256:(k+1)*256],
                            wa_sb[:, o*128:(o+1)*128],
                            z1tiles[(r, o)][:, gp*256:(gp+1)*256])
                    # evac: z2 free = b*32 + o, one [128,1024] copy
                    dst = z2_slabs[gp][:].rearrange("q (b o) -> q b o",
                                                    o=32)[:, :, o4*4:o4*4+4]
                    src = ps_a[:].rearrange("q (k b) -> q b k", b=256)
                    nc.scalar.copy(dst, src)
                    drain_credit += 2.0
                    while drain_credit >= 1.0 and tails:
                        tails.pop(0)()
                        drain_credit -= 1.0
            tails.extend([p for g in range(NGRP)
                          for p in make_tail(r, g, z2_slabs[g // 2])])
        # drain the last rep's tails
        for p in tails:
            p()

    nc.compile()
    return nc


def _get_program():
    if "nc" not in _cache:
        _cache["nc"] = _build_program()
    return _cache["nc"]


# ---------------------------------------------------------------- entry
def kernel(x, angles):
    import ml_dtypes
    x = np.asarray(x, dtype=np.float32)
    angles = np.asarray(angles, dtype=np.float32)
    assert x.shape == (BATCH, DIM) and angles.shape == (LAYERS, DIM // 2)

    WA, WB = _build_weights(angles)
    wa_bf = WA.astype(ml_dtypes.bfloat16)
    wb_bf = WB.astype(ml_dtypes.bfloat16)
    x_bf = x.astype(ml_dtypes.bfloat16)
    ident = np.eye(128, dtype=np.float32)
    nc = _get_program()

    in_maps = []
    for core in range(N_CORES):
        in_maps.append({
            "x": np.ascontiguousarray(x_bf[core * BC:(core + 1) * BC]),
            "wa": wa_bf, "wb": wb_bf, "ident": ident,
        })
    res = bass_utils.run_bass_kernel_spmd(
        nc, in_maps, core_ids=list(range(N_CORES)))
    out = np.concatenate([r["out"] for r in res.results], axis=0)
    return out.astype(np.float32)
